# revision 1
# baseline (speedup 1.0000x reference)
"""GAttentionBlock (GroupNorm + 8-head self-attention + proj + residual) on 8
Trainium2 NeuronCores, data-parallel over the batch dimension (B=8 -> 1 image
per core).

Per-core pipeline (all matmuls bf16 with fp32 PSUM accumulation):
  A) GroupNorm(32 groups): per-channel bn_stats; group reduce/broadcast across
     partitions via tiny PE matmuls against host-constant 0/1 indicator
     matrices; rstd as exp(-0.5*ln(var+eps)) to stay in the exp/ln ACT table
     set; fused apply xn = x*A + B (per-channel scalars), bf16 out.
  B) q,k per head in [80(d), T] head-aligned tiles; v computed directly
     TRANSPOSED (vT[s, c]) with xn chunks as the stationary operand; a
     constant ones-column at col 96 of each AV weight tile makes the AV
     matmul also emit the softmax denominator at psum partition 96
     (32-aligned, hence legally readable).
  C) Per head: scores S_T[s,t] = k.T q (K=80); exp via one ACT instruction
     per 2-bank psum tile (no max subtraction: scores ~ N(0,1)); AV
     accumulation per 512-half so the denominator reciprocal + bf16
     DRAM-bounce partition-broadcast + normalize overlap the other half.
  D) proj with per-head split-K accumulation, + bias + fp32 residual.

Scheduling: PE executes its stream in order, so stages B and C are software-
pipelined across heads -- each steady-state iteration emits
AV+normalize(h-1) | scores(h) | qkv-matmuls(h+1), sized so PE work covers
ACT's exp(h); vT is produced in two 4-head groups inside the pipeline. This
took the measured kernel from 250us to ~165us (TimelineSim 164us; For_i
loop-slope 220us including ~50us/iter loop artifacts). PSUM budget: scores
2x2 banks + AV 2x1 + work 2x1 = 8 banks exactly. Engine busy (cost model):
PE 117us (582 matmuls -- the floor for this decomposition given N<=512,
K<=128, and the 32-aligned-partition-base rule), ACT 77us, DVE 80us.
"""
import copy

import numpy as np
import ml_dtypes

import concourse.bass as bass
import concourse.mybir as mybir
import concourse.tile as tile
from concourse.bass_utils import run_bass_kernel_spmd

F32 = mybir.dt.float32
BF16 = mybir.dt.bfloat16

B, C, HH, WW = 8, 640, 32, 32
T = HH * WW            # 1024
NH, D = 8, 80          # heads, head dim
G = 32                 # groupnorm groups
GS = C // G            # 20 channels per group
EPS = 1e-5
NCHUNK = C // 128      # 5 channel chunks of 128
NSC = T // 128         # 8 sequence chunks of 128
SCALE = 1.0 / np.sqrt(np.float64(D))  # applied inside exp

_MAXW = 1


def _split_multiwait(nc):
    """This walrus build rejects >1 sync-wait command per instruction. Move
    extra waits onto same-engine NoOps inserted just before the instruction."""
    ctr = 0
    new_module = copy.replace(nc.m, functions=[])
    for function in nc.m.functions:
        new_function = copy.replace(function, blocks=[])
        new_function.set_allocations_from_list(function.allocations)
        for block in function.blocks:
            new_insts = []
            for inst in block.instructions:
                si = inst.sync_info
                ow = list(si.on_wait) if (si is not None and si.on_wait) else []
                if len(ow) > _MAXW:
                    head, tail = ow[:-_MAXW], ow[-_MAXW:]
                    for w in head:
                        ctr += 1
                        new_insts.append(mybir.InstNoOp(
                            name=f"mwsplit_{ctr}",
                            engine=inst.engine,
                            sync_info=mybir.SyncInfo(on_wait=[w], on_update=[]),
                            bass_nofuse=True,
                        ))
                    inst.sync_info = mybir.SyncInfo(
                        on_wait=tail,
                        on_update=list(si.on_update) if si.on_update else [],
                    )
                new_insts.append(inst)
            new_function.blocks.append(copy.replace(block, instructions=new_insts))
        new_module.functions.append(new_function)
    nc.m = new_module


def _build_program(repeat=1, loop_n=0):
    nc = bass.Bass("TRN2", target_bir_lowering=False, num_devices=8)

    x_d = nc.dram_tensor("x", [C, T], F32, kind="ExternalInput").ap()
    wqkvT_d = nc.dram_tensor("wqkvT", [C, 3 * C], BF16, kind="ExternalInput").ap()
    pwT_d = nc.dram_tensor("pwT", [D, NH, C], BF16, kind="ExternalInput").ap()
    qkb_d = nc.dram_tensor("qkb", [D, 2 * NH], F32, kind="ExternalInput").ap()
    vb_d = nc.dram_tensor("vb", [C], F32, kind="ExternalInput").ap()
    nw_d = nc.dram_tensor("nw", [C], F32, kind="ExternalInput").ap()
    nb_d = nc.dram_tensor("nb", [C], F32, kind="ExternalInput").ap()
    pb_d = nc.dram_tensor("pb", [C], F32, kind="ExternalInput").ap()
    ind1_d = nc.dram_tensor("ind1", [C, G], F32, kind="ExternalInput").ap()
    ind2_d = nc.dram_tensor("ind2", [G, C], F32, kind="ExternalInput").ap()
    o_d = nc.dram_tensor("o", [C, T], F32, kind="ExternalOutput").ap()

    x_dv = x_d.rearrange("(o p) t -> p o t", p=128)       # [128, 5, 1024]
    o_dv = o_d.rearrange("(o p) t -> p o t", p=128)

    with tile.TileContext(nc) as tc:
        with tc.tile_pool(name="wpool", bufs=1) as wp, \
             tc.tile_pool(name="data", bufs=1) as dp, \
             tc.tile_pool(name="ptile", bufs=2) as pp, \
             tc.tile_pool(name="small", bufs=2) as sp, \
             tc.tile_pool(name="ps", bufs=2, space="PSUM") as ps, \
             tc.tile_pool(name="dram", bufs=2, space="DRAM") as dr:

            # ---------- weight / constant loads ----------
            wqkvT = wp.tile([128, NCHUNK, 3 * C], BF16)
            nc.sync.dma_start(out=wqkvT, in_=wqkvT_d.rearrange("(o p) n -> p o n", p=128))
            pwT = wp.tile([D, NH, C], BF16)
            nc.sync.dma_start(out=pwT, in_=pwT_d)
            qkb = wp.tile([D, 2 * NH], F32)
            nc.sync.dma_start(out=qkb, in_=qkb_d)
            nwb = wp.tile([128, NCHUNK, 2], F32)
            nc.sync.dma_start(out=nwb[:, :, 0], in_=nw_d.rearrange("(o p) -> p o", p=128))
            nc.sync.dma_start(out=nwb[:, :, 1], in_=nb_d.rearrange("(o p) -> p o", p=128))
            pb = wp.tile([128, NCHUNK], F32)
            nc.sync.dma_start(out=pb, in_=pb_d.rearrange("(o p) -> p o", p=128))
            vbb = wp.tile([128, C], F32)
            nc.sync.dma_start(out=vbb, in_=vb_d[None, :].to_broadcast([128, C]))

            # group indicator matrices (host-constant inputs)
            ind1 = wp.tile([128, NCHUNK, G], F32)   # [channel -> group] one-hot
            ind2 = wp.tile([G, NCHUNK, 128], F32)   # [group -> channel] one-hot
            nc.sync.dma_start(out=ind1, in_=ind1_d.rearrange("(o p) g -> p o g", p=128))
            nc.sync.dma_start(out=ind2, in_=ind2_d.rearrange("g (o p) -> g o p", p=128))

            eps_t = wp.tile([G, 1], F32)
            nc.vector.memset(eps_t, EPS)

            import contextlib
            loop_cm = tc.For_i(0, loop_n, 1) if loop_n else contextlib.nullcontext()
            with loop_cm:
              for _rep in range(repeat):
                # ---------- stage A: load x + GroupNorm ----------
                x_sb = dp.tile([128, NCHUNK, T], F32)
                for j in range(NCHUNK):
                    nc.sync.dma_start(out=x_sb[:, j, :], in_=x_dv[:, j, :])

                stats = sp.tile([128, 2, 6], F32, tag="gn_stats")
                ss = dp.tile([128, NCHUNK, 2], F32)    # per-channel [mean, E[x^2]]
                for j in range(NCHUNK):
                    nc.vector.bn_stats(out=stats[:, 0, :], in_=x_sb[:, j, 0:512])
                    nc.vector.bn_stats(out=stats[:, 1, :], in_=x_sb[:, j, 512:1024])
                    nc.vector.bn_aggr(out=ss[:, j, :], in_=stats)
                    # ss[...,1] currently var; make it var + mean^2 = E[x^2]
                    nc.vector.tensor_tensor(out=stats[:, 0, 0:1], in0=ss[:, j, 0:1],
                                            in1=ss[:, j, 0:1], op=mybir.AluOpType.mult)
                    nc.vector.tensor_tensor(out=ss[:, j, 1:2], in0=ss[:, j, 1:2],
                                            in1=stats[:, 0, 0:1], op=mybir.AluOpType.add)

                ps_g = ps.tile([G, 2], F32, tag="work")
                for j in range(NCHUNK):
                    nc.tensor.matmul(ps_g, lhsT=ind1[:, j, :], rhs=ss[:, j, :],
                                     start=(j == 0), stop=(j == NCHUNK - 1))
                # group stats -> mean_g, rstd_g
                gm = sp.tile([G, 2], F32, tag="gn_gm")       # [mean_g, rstd_g]
                tmp_g = sp.tile([G, 2], F32, tag="gn_tmp")
                nc.vector.tensor_scalar_mul(gm, ps_g, 1.0 / GS)           # [mean, E2]
                nc.vector.tensor_tensor(out=tmp_g[:, 0:1], in0=gm[:, 0:1],
                                        in1=gm[:, 0:1], op=mybir.AluOpType.mult)
                nc.vector.tensor_tensor(out=tmp_g[:, 1:2], in0=gm[:, 1:2],
                                        in1=tmp_g[:, 0:1], op=mybir.AluOpType.subtract)
                nc.scalar.activation(out=tmp_g[:, 1:2], in_=tmp_g[:, 1:2],
                                     func=mybir.ActivationFunctionType.Ln,
                                     bias=eps_t, scale=1.0)
                nc.scalar.activation(out=gm[:, 1:2], in_=tmp_g[:, 1:2],
                                     func=mybir.ActivationFunctionType.Exp,
                                     scale=-0.5)   # rstd_g = (var+eps)^-0.5

                xn = dp.tile([128, NCHUNK, T], BF16)
                ab = dp.tile([128, NCHUNK, 2], F32)
                for j in range(NCHUNK):
                    ps_bc = ps.tile([128, 2], F32, tag="work", name=f"ps_bc{j}")
                    nc.tensor.matmul(ps_bc, lhsT=ind2[:, j, :], rhs=gm,
                                     start=True, stop=True)
                    # A = rstd_c * norm_w ; B = norm_b - mean_c * A
                    nc.vector.tensor_tensor(out=ab[:, j, 0:1], in0=ps_bc[:, 1:2],
                                            in1=nwb[:, j, 0:1], op=mybir.AluOpType.mult)
                    nc.vector.tensor_tensor(out=ab[:, j, 1:2], in0=ps_bc[:, 0:1],
                                            in1=ab[:, j, 0:1], op=mybir.AluOpType.mult)
                    nc.vector.tensor_tensor(out=ab[:, j, 1:2], in0=nwb[:, j, 1:2],
                                            in1=ab[:, j, 1:2], op=mybir.AluOpType.subtract)
                    nc.vector.tensor_scalar(out=xn[:, j, :], in0=x_sb[:, j, :],
                                            scalar1=ab[:, j, 0:1], scalar2=ab[:, j, 1:2],
                                            op0=mybir.AluOpType.mult,
                                            op1=mybir.AluOpType.add)

                # ---------- stages B+C: software-pipelined qkv + attention ----------
                # PE stream per head-iteration: scores(h) | qk(h+2) | AV(h-1),
                # sized to overlap ACT's exp(h). vT emitted in two 4-head
                # chunks. All psum->sbuf copies on DVE (ACT is the C-stage
                # bottleneck).
                q_sb = dp.tile([D, NH, T], BF16)
                k_sb = dp.tile([D, NH, T], BF16)
                vT = dp.tile([128, NSC, NH, 97], BF16)
                a_sb = dp.tile([D, NH, T], BF16)
                nc.vector.memset(vT[:, :, :, D:96], 0.0)
                nc.vector.memset(vT[:, :, :, 96:97], 1.0)
                p_tiles = {}

                def emit_qk(h):
                    for w in range(2):  # 0=q, 1=k
                        jt = w * NH + h
                        dst = q_sb if w == 0 else k_sb
                        for tt in range(2):
                            ps_qk = ps.tile([D, 512], F32, tag="work",
                                            name=f"ps_qk{h}_{w}_{tt}")
                            for j in range(NCHUNK):
                                nc.tensor.matmul(
                                    ps_qk,
                                    lhsT=wqkvT[:, j, jt * D:(jt + 1) * D],
                                    rhs=xn[:, j, tt * 512:(tt + 1) * 512],
                                    start=(j == 0), stop=(j == NCHUNK - 1))
                            nc.vector.tensor_scalar(
                                out=dst[:, h, tt * 512:(tt + 1) * 512], in0=ps_qk,
                                scalar1=qkb[:, jt:jt + 1], scalar2=None,
                                op0=mybir.AluOpType.add)

                def emit_vT(nn):  # nn selects a 4-head group
                    for sc in range(NSC):
                        ps_v = ps.tile([128, 320], F32, tag="work",
                                       name=f"ps_v{sc}_{nn}")
                        for j in range(NCHUNK):
                            nc.tensor.matmul(
                                ps_v,
                                lhsT=xn[:, j, sc * 128:(sc + 1) * 128],
                                rhs=wqkvT[:, j, 2 * C + nn * 320: 2 * C + (nn + 1) * 320],
                                start=(j == 0), stop=(j == NCHUNK - 1))
                        nc.vector.tensor_tensor(
                            out=vT[:, sc, nn * 4:(nn + 1) * 4, 0:D],
                            in0=ps_v.rearrange("p (h d) -> p h d", h=4),
                            in1=vbb[:, nn * 320:(nn + 1) * 320].rearrange(
                                "p (h d) -> p h d", h=4),
                            op=mybir.AluOpType.add)

                def emit_scores_exp(h):
                    p_t = pp.tile([128, NSC, T], BF16, tag="probs", name=f"p_t{h}")
                    p_tiles[h] = p_t
                    for sc in range(NSC):
                        ps_s = ps.tile([128, T], F32, tag="scores",
                                       name=f"ps_s{h}_{sc}", bufs=2)
                        for tt in range(2):
                            nc.tensor.matmul(
                                ps_s[:, tt * 512:(tt + 1) * 512],
                                lhsT=k_sb[:, h, sc * 128:(sc + 1) * 128],
                                rhs=q_sb[:, h, tt * 512:(tt + 1) * 512],
                                start=True, stop=True)
                        nc.scalar.activation(out=p_t[:, sc, :], in_=ps_s,
                                             func=mybir.ActivationFunctionType.Exp,
                                             scale=float(SCALE))

                def emit_av_norm(h):
                    p_t = p_tiles.pop(h)
                    rinv = sp.tile([1, T], BF16, tag="rinv", name=f"rinv{h}")
                    r_dr = dr.tile([1, T], BF16, tag="rbounce", name=f"r_dr{h}")
                    rb = sp.tile([D, T], BF16, tag="rb", name=f"rb{h}")
                    for tt in range(2):
                        sl = slice(tt * 512, (tt + 1) * 512)
                        ps_a = ps.tile([97, 512], F32, tag="av",
                                       name=f"ps_a{h}_{tt}", bufs=2)
                        for sc in range(NSC):
                            nc.tensor.matmul(
                                ps_a,
                                lhsT=vT[:, sc, h, :],
                                rhs=p_t[:, sc, sl],
                                start=(sc == 0), stop=(sc == NSC - 1))
                        with nc.allow_low_precision(reason="softmax denom bf16"):
                            nc.vector.reciprocal(out=rinv[0:1, sl], in_=ps_a[96:97, :])
                        nc.sync.dma_start(out=r_dr[:, sl], in_=rinv[:, sl])
                        nc.sync.dma_start(out=rb[:, sl],
                                          in_=r_dr[0:1, sl].to_broadcast([D, 512]))
                        nc.vector.tensor_tensor(out=a_sb[:, h, sl],
                                                in0=ps_a[0:D, :], in1=rb[:, sl],
                                                op=mybir.AluOpType.mult)

                emit_qk(0)
                emit_scores_exp(0)
                emit_qk(1)
                emit_vT(0)
                for h in range(1, NH):
                    emit_av_norm(h - 1)
                    emit_scores_exp(h)
                    if h + 1 < NH:
                        emit_qk(h + 1)
                    if h == 2:
                        emit_vT(1)
                emit_av_norm(NH - 1)

                # ---------- stage D: proj + bias + residual ----------
                for j in range(NCHUNK):
                    for tt in range(2):
                        ps_p = ps.tile([128, 512], F32, tag="work", name=f"ps_p{j}_{tt}")
                        for h in range(NH):
                            nc.tensor.matmul(
                                ps_p,
                                lhsT=pwT[:, h, j * 128:(j + 1) * 128],
                                rhs=a_sb[:, h, tt * 512:(tt + 1) * 512],
                                start=(h == 0), stop=(h == NH - 1))
                        out_t = sp.tile([128, 512], F32, tag="out")
                        nc.scalar.activation(out=out_t, in_=ps_p,
                                             func=mybir.ActivationFunctionType.Identity,
                                             bias=pb[:, j:j + 1], scale=1.0)
                        nc.vector.tensor_tensor(out=out_t, in0=out_t,
                                                in1=x_sb[:, j, tt * 512:(tt + 1) * 512],
                                                op=mybir.AluOpType.add)
                        nc.sync.dma_start(out=o_dv[:, j, tt * 512:(tt + 1) * 512],
                                          in_=out_t)

    _split_multiwait(nc)
    return nc


_NC_CACHE = {}


def _get_program(repeat=1, loop_n=0):
    key = (repeat, loop_n)
    if key not in _NC_CACHE:
        _NC_CACHE[key] = _build_program(repeat, loop_n)
    return _NC_CACHE[key]


def _prep_shared(norm_w, norm_b, qkv_w, qkv_b, proj_w, proj_b):
    qkv_w = np.asarray(qkv_w, dtype=np.float32)
    proj_w = np.asarray(proj_w, dtype=np.float32)
    qkv_b = np.asarray(qkv_b, dtype=np.float32)
    wqkvT = np.ascontiguousarray(
        qkv_w.reshape(3, NH, D, C).transpose(3, 0, 1, 2).reshape(C, 3 * C)
    ).astype(ml_dtypes.bfloat16)
    pwT = np.ascontiguousarray(
        proj_w.reshape(C, NH, D).transpose(2, 1, 0)
    ).astype(ml_dtypes.bfloat16)
    qkb = np.ascontiguousarray(
        qkv_b.reshape(3, NH, D)[:2].transpose(2, 0, 1).reshape(D, 2 * NH)
    ).astype(np.float32)
    vb = np.ascontiguousarray(qkv_b[2 * C:]).astype(np.float32)
    cidx = np.arange(C) // GS
    ind1 = np.zeros((C, G), dtype=np.float32)
    ind1[np.arange(C), cidx] = 1.0
    ind2 = np.ascontiguousarray(ind1.T)
    return {
        "ind1": ind1,
        "ind2": ind2,
        "wqkvT": wqkvT,
        "pwT": pwT,
        "qkb": qkb,
        "vb": vb,
        "nw": np.ascontiguousarray(np.asarray(norm_w, dtype=np.float32)),
        "nb": np.ascontiguousarray(np.asarray(norm_b, dtype=np.float32)),
        "pb": np.ascontiguousarray(np.asarray(proj_b, dtype=np.float32)),
    }


def make_in_maps(x, norm_w, norm_b, qkv_w, qkv_b, proj_w, proj_b):
    x = np.asarray(x, dtype=np.float32)
    shared = _prep_shared(norm_w, norm_b, qkv_w, qkv_b, proj_w, proj_b)
    xs = x.reshape(B, C, T)
    return [dict(shared, x=np.ascontiguousarray(xs[i])) for i in range(B)]


def kernel(x, norm_w, norm_b, qkv_w, qkv_b, proj_w, proj_b):
    nc = _get_program()
    in_maps = make_in_maps(x, norm_w, norm_b, qkv_w, qkv_b, proj_w, proj_b)
    res = run_bass_kernel_spmd(nc, in_maps, core_ids=list(range(B)), trace=False)
    out = np.stack([res.results[i]["o"].reshape(C, HH, WW) for i in range(B)])
    return out.astype(np.float32)

